# revision 29
# baseline (speedup 1.0000x reference)
"""Memory-efficient Dice loss on 8 Trainium2 NeuronCores.

Full inputs:
  logits  (2, 16, 64, 128, 128) fp32
  targets (2, 64, 128, 128) int64  (values 0..15)
Output: scalar fp32 loss = 1 - mean_{b, c != 0} dice[b, c].

Sharding: 8 cores over (B=2) x (D quartered into 4 slabs of 16).
Each core reduces its shard to one 128x128 stats matrix; host combines
the tiny per-core stats and applies the dice formula.

Per-core math (voxels n, classes c), fp16 on-chip:
  e[n,c] = exp(logit[n,c]); Z[n] = sum_c e; r[n] = 1/Z
  R slot c>=1: (t==c) * r;  R slot 0: plain r
  PSUM-accumulated fp16 matmuls: S[c1,c2] = sum_n e[n,c1] * R[n,c2]
    diag(S)[c>=1]        = intersection
    S[:, 0]              = sum_n e_c1 * r = probs_sum  (exact column)
    sum_c1 S[c1, c>=1]   = sum_n (Z*r) * mask_c = counts  (Z*r == 1)
  Class 0 is dice-excluded (IGNORE_INDEX), so its mask is never needed.

Layout: "blocked chunk-major". E/R tiles hold element (chunk m, slot c,
lane g) at m*128 + c*8 + g: each of the 64 matmuls per block reads a
CONTIGUOUS 128-column slice (walrus requires 1-free-dim matmul
operands) and every elementwise op sees packed 8-lane fp16 runs.

Engine facts (HW-measured): DVE TT adds run 2x (0.56ns/elem); DVE
scalar_tensor_tensor gets NO fast mode (~600ns per 512-elem op, fp32
in1 free); GpSimd TT ops starve DVE when run concurrently (4-7x DVE
slowdown), so the whole per-block chain stays on DVE and GpSimd only
issues DMAs on its ring. PE overlaps LDWEIGHTS with MATMUL (~107ns per
128-col fp16 matmul). ACT exp runs 0.87ns/elem with 8-lane writes.

DMA: host pre-permutes logits to [nblk][p][c][j] fp16 (the kernel is
fp16 internally anyway, so the cast costs no accuracy headroom: final
rel err stays ~1e-4, tolerance is 2e-2) and targets to
[nblk][p][j] fp16; each block is two contiguous class-half dma_starts
split across the sync/gpsimd rings (block 0: quarters, to cut the
pipeline head). The last block's mask ops + matmuls are split in
m-halves so the PE drain starts at the half mark.
"""

import numpy as np

import concourse.bass as bass
import concourse.mybir as mybir
import concourse.tile as tile
from concourse import bacc
from concourse.bass_utils import run_bass_kernel_spmd

B, C, D, H, W = 2, 16, 64, 128, 128
P = 128            # SBUF partitions
NCORES = 8
DSH = D // 4       # d-planes per core
N = DSH * H * W    # voxels per core = 262144
G = 8              # packed chunk lanes per matmul
MOUT = C * G       # 128

NBLK = 4
T = N // (P * NBLK)         # voxel columns per block = 512
NMM = T // G                # matmuls per block = 64

SMOOTH = 1.0
IGNORE_INDEX = 0


def build():
    """Build the SPMD single-core Bass program."""
    fp32 = mybir.dt.float32
    fp16 = mybir.dt.float16
    AL = mybir.AluOpType
    Act = mybir.ActivationFunctionType

    nc = bacc.Bacc("TRN2", target_bir_lowering=False, debug=False)
    logits_d = nc.dram_tensor("logits", [NBLK, P, C * T], fp16, kind="ExternalInput")
    targets_d = nc.dram_tensor("targets", [NBLK, P, T], fp16, kind="ExternalInput")
    icb_d = nc.dram_tensor("icb", [P, NMM * MOUT], fp16, kind="ExternalInput")
    out_d = nc.dram_tensor("out", [2, MOUT, MOUT], fp32, kind="ExternalOutput")

    def body(tc, pools):
        lpool, tpool, epool, rpool, zpool, fpool, psump, fin, cpool = pools
        # iota-constant tile ICB[m, c, g] = c for the bulk is_eq masks,
        # DMA'd from the host (keeps DVE free of init memsets)
        icb = cpool.tile([P, NMM * MOUT], fp16)
        icb4 = icb[:].rearrange("p (m c g) -> p m c g", m=NMM, c=C)
        # two PSUM accumulators (even/odd blocks) so the first copy+DMA
        # overlaps the last block's compute
        accs = [psump.tile([MOUT, MOUT], fp32, name=f"acc{i}") for i in range(2)]
        outs = [fin.tile([MOUT, MOUT], fp32, name=f"outs{i}") for i in range(2)]
        for blk in range(NBLK):
            first, last = blk == 0, blk == NBLK - 1
            Lb = lpool.tile([P, C * T], fp16, tag="L")
            tt = tpool.tile([P, T], fp16, tag="t")
            ring_a = nc.sync if blk % 2 == 0 else nc.gpsimd
            ring_b = nc.gpsimd if blk % 2 == 0 else nc.sync
            la = logits_d.ap()[blk]
            # block 0: quarter DMAs/EXPs so the first exp starts sooner
            nq = 4 if first else 2
            QC = C // nq
            for q in range(nq):
                ring = ring_a if q % 2 == 0 else ring_b
                ring.dma_start(
                    Lb[:, q * QC * T : (q + 1) * QC * T],
                    la[:, q * QC * T : (q + 1) * QC * T],
                )
            ring_a.dma_start(tt[:], targets_d.ap()[blk])
            if first:
                ring_b.dma_start(icb[:], icb_d.ap())

            E = epool.tile([P, NMM * MOUT], fp16, tag="E")
            R = rpool.tile([P, NMM * MOUT], fp16, tag="R")
            zt = zpool.tile([P, 8 * T], fp16, tag="zt")
            Zf = fpool.tile([P, T], fp32, tag="Zf")
            Rf = fpool.tile([P, T], fp32, tag="Rf")
            rc = fpool.tile([P, T], fp16, tag="rc")
            E4 = E[:].rearrange("p (m c g) -> p m c g", m=NMM, c=C)
            R4 = R[:].rearrange("p (m c g) -> p m c g", m=NMM, c=C)
            Lg = Lb[:].rearrange("p (c m g) -> p c m g", c=C, g=G)
            tt4 = tt[:].rearrange("p (m o g) -> p m o g", o=1, g=G)
            rc3 = rc[:].rearrange("p (m g) -> p m g", g=G)
            rc4 = rc[:].rearrange("p (m o g) -> p m o g", o=1, g=G)
            z3 = zt[:].rearrange("p (s j) -> p s j", s=8)
            zg = zt[:].rearrange("p (s m g) -> p s m g", s=8, g=G)

            # e = exp(logits), one op per DMA granule
            for q in range(nq):
                nc.scalar.activation(
                    E4[:, :, q * QC : (q + 1) * QC, :].rearrange(
                        "p m c g -> p c m g"
                    ),
                    Lg[:, q * QC : (q + 1) * QC],
                    Act.Exp,
                )

            # Z tree fully on DVE (2x fp16 adds), fp32 tail for recip
            nc.vector.tensor_tensor(
                zg[:, 0:4],
                E4[:, :, 0:4, :].rearrange("p m s g -> p s m g"),
                E4[:, :, 4:8, :].rearrange("p m s g -> p s m g"),
                AL.add,
            )
            nc.vector.tensor_tensor(
                zg[:, 4:8],
                E4[:, :, 8:12, :].rearrange("p m s g -> p s m g"),
                E4[:, :, 12:16, :].rearrange("p m s g -> p s m g"),
                AL.add,
            )
            nc.vector.tensor_tensor(
                z3[:, 0:4, :], z3[:, 0:4, :], z3[:, 4:8, :], AL.add
            )
            nc.vector.tensor_tensor(
                z3[:, 0:2, :], z3[:, 0:2, :], z3[:, 2:4, :], AL.add
            )
            nc.vector.tensor_tensor(Zf[:], z3[:, 0, :], z3[:, 1, :], AL.add)
            nc.vector.reciprocal_approx_fast(Rf[:], Zf[:])
            nc.vector.tensor_copy(rc[:], Rf[:])

            # last block: split mask work + matmuls in m-halves so the
            # PE drain starts at the half mark
            acc = accs[blk % 2]
            mranges = [(0, NMM // 2), (NMM // 2, NMM)] if last else [(0, NMM)]
            for m0, m1 in mranges:
                ms = slice(m0, m1)
                # R slot 0 := plain r (probs_sum column)
                nc.vector.tensor_copy(R4[:, ms, 0, :], rc3[:, ms])
                # bulk masks: one 2x is_eq vs the iota tile, one 2x
                # broadcast multiply folds r in (both HW-verified 2x)
                tgt = R4[:, ms, 1:C, :]
                tin, _ = bass.broadcast_tensor_aps(tt4[:, ms], tgt)
                rin, _ = bass.broadcast_tensor_aps(rc4[:, ms], tgt)
                nc.vector.tensor_tensor(tgt, tin, icb4[:, ms, 1:C, :], AL.is_equal)
                nc.vector.tensor_tensor(tgt, tgt, rin, AL.mult)
                for m in range(m0, m1):
                    nc.tensor.matmul(
                        acc[:],
                        E[:, m * MOUT : (m + 1) * MOUT],
                        R[:, m * MOUT : (m + 1) * MOUT],
                        start=(blk < 2 and m == 0),
                        stop=(blk >= 2 and m == NMM - 1),
                    )
            if blk >= 2:
                # accumulator for this parity is complete: copy + ship
                nc.vector.tensor_copy(outs[blk % 2][:], acc[:])
                nc.sync.dma_start(out_d.ap()[blk % 2], outs[blk % 2][:])

    with tile.TileContext(nc) as tc:
        with (
            tc.tile_pool(name="lpool", bufs=2) as lpool,
            tc.tile_pool(name="tpool", bufs=2) as tpool,
            tc.tile_pool(name="epool", bufs=3) as epool,
            tc.tile_pool(name="rpool", bufs=3) as rpool,
            tc.tile_pool(name="zpool", bufs=2) as zpool,
            tc.tile_pool(name="fpool", bufs=2) as fpool,
            tc.tile_pool(name="psum", bufs=1, space="PSUM") as psump,
            tc.tile_pool(name="fin", bufs=1) as fin,
            tc.tile_pool(name="cpool", bufs=1) as cpool,
        ):
            body(tc, (lpool, tpool, epool, rpool, zpool, fpool, psump, fin, cpool))
    nc.compile()
    return nc


_NC_CACHE = {}


def _get_nc():
    if "nc" not in _NC_CACHE:
        _NC_CACHE["nc"] = build()
    return _NC_CACHE["nc"]


def stats_from_out(out_mat):
    """out[c1*8+g, c2*8+g] summed over g -> one 16x16 stats matrix."""
    M = out_mat.astype(np.float64).reshape(C, G, C, G)
    return np.einsum("agbg->ab", M)


def loss_from_stats(S_per_b):
    """S_per_b: (B, 16, 16) combined stats -> scalar loss.

    R slot 0 held plain r, slots 1..15 held (t==c)*r, so:
      probs_sum[c] = S[c, 0], counts[c] = sum_c1 S[c1, c] (c >= 1),
      inter[c] = S[c, c] (c >= 1). Class 0 is dice-excluded.
    """
    idx = np.arange(C)
    inter = S_per_b[:, idx, idx]          # (B, C); [*, 0] is garbage
    probs_sum = S_per_b[:, :, 0]          # (B, C)  sum_n e_c * r
    counts = S_per_b.sum(axis=1)          # (B, C); [*, 0] is garbage
    dice = (2.0 * inter + SMOOTH) / (probs_sum + counts + SMOOTH)
    mask = np.ones(C)
    mask[IGNORE_INDEX] = 0.0
    mean_dice = (dice * mask[None, :]).sum() / (B * (C - 1))
    return np.float32(1.0 - mean_dice)


def shard_inputs(logits, targets):
    """Core i gets batch i//4, d-slab i%4.

    Device layout (voxel n = p*(NBLK*T) + blk*T + j):
      logits  [NBLK, P, C, T] fp32
      targets [NBLK, P, T]    fp16
    """
    icb_row = np.broadcast_to(
        np.arange(C, dtype=np.float16)[None, :, None], (NMM, C, G)
    ).reshape(-1)
    icb_arr = np.ascontiguousarray(np.broadcast_to(icb_row[None, :], (P, NMM * MOUT)))
    in_maps = []
    for i in range(NCORES):
        b, q = divmod(i, 4)
        lg = logits[b, :, q * DSH : (q + 1) * DSH].reshape(C, P, NBLK, T)
        lg = np.ascontiguousarray(lg.transpose(2, 1, 0, 3), dtype=np.float16)
        tg = targets[b, q * DSH : (q + 1) * DSH].reshape(P, NBLK, T)
        tg = np.ascontiguousarray(tg.transpose(1, 0, 2)).astype(np.float16)
        in_maps.append(
            {"logits": lg.reshape(NBLK, P, C * T), "targets": tg, "icb": icb_arr}
        )
    return in_maps


def kernel(logits, targets):
    logits = np.asarray(logits)
    targets = np.asarray(targets)
    nc = _get_nc()
    in_maps = shard_inputs(logits, targets)
    res = run_bass_kernel_spmd(nc, in_maps, list(range(NCORES))).results
    S = np.zeros((B, C, C), np.float64)
    for i in range(NCORES):
        om = res[i]["out"]
        S[i // 4] += stats_from_out(om[0]) + stats_from_out(om[1])
    return loss_from_stats(S)


# revision 30
# speedup vs baseline: 1.0775x; 1.0775x over previous
"""Memory-efficient Dice loss on 8 Trainium2 NeuronCores.

Full inputs:
  logits  (2, 16, 64, 128, 128) fp32
  targets (2, 64, 128, 128) int64  (values 0..15)
Output: scalar fp32 loss = 1 - mean_{b, c != 0} dice[b, c].

Sharding: 8 cores over (B=2) x (D quartered into 4 slabs of 16).
Each core reduces its shard to one 128x128 stats matrix; host combines
the tiny per-core stats and applies the dice formula.

Per-core math (voxels n, classes c), fp16 on-chip:
  e[n,c] = exp(logit[n,c]); Z[n] = sum_c e; r[n] = 1/Z
  R slot c>=1: (t==c) * r;  R slot 0: plain r
  PSUM-accumulated fp16 matmuls: S[c1,c2] = sum_n e[n,c1] * R[n,c2]
    diag(S)[c>=1]        = intersection
    S[:, 0]              = sum_n e_c1 * r = probs_sum  (exact column)
    sum_c1 S[c1, c>=1]   = sum_n (Z*r) * mask_c = counts  (Z*r == 1)
  Class 0 is dice-excluded (IGNORE_INDEX), so its mask is never needed.

Layout: "blocked chunk-major". E/R tiles hold element (chunk m, slot c,
lane g) at m*128 + c*8 + g: each of the 64 matmuls per block reads a
CONTIGUOUS 128-column slice (walrus requires 1-free-dim matmul
operands) and every elementwise op sees packed 8-lane fp16 runs.

Engine facts (HW-measured): DVE TT adds run 2x (0.56ns/elem); DVE
scalar_tensor_tensor gets NO fast mode (~600ns per 512-elem op, fp32
in1 free); GpSimd TT ops starve DVE when run concurrently (4-7x DVE
slowdown), so the whole per-block chain stays on DVE and GpSimd only
issues DMAs on its ring. PE overlaps LDWEIGHTS with MATMUL (~107ns per
128-col fp16 matmul). ACT exp runs 0.87ns/elem with 8-lane writes.

DMA: host pre-permutes logits to [nblk][p][c][j] fp16 (the kernel is
fp16 internally anyway, so the cast costs no accuracy headroom: final
rel err stays ~1e-4, tolerance is 2e-2) and targets to
[nblk][p][j] fp16; each block is two contiguous class-half dma_starts
split across the sync/gpsimd rings (block 0: quarters, to cut the
pipeline head). The last block's mask ops + matmuls are split in
m-halves so the PE drain starts at the half mark.
"""

import numpy as np

import concourse.bass as bass
import concourse.mybir as mybir
import concourse.tile as tile
from concourse import bacc
from concourse.bass_utils import run_bass_kernel_spmd

B, C, D, H, W = 2, 16, 64, 128, 128
P = 128            # SBUF partitions
NCORES = 8
DSH = D // 4       # d-planes per core
N = DSH * H * W    # voxels per core = 262144
G = 8              # packed chunk lanes per matmul
MOUT = C * G       # 128

NBLK = 4
T = N // (P * NBLK)         # voxel columns per block = 512
NMM = T // G                # matmuls per block = 64

SMOOTH = 1.0
IGNORE_INDEX = 0


def build():
    """Build the SPMD single-core Bass program."""
    fp32 = mybir.dt.float32
    fp16 = mybir.dt.float16
    AL = mybir.AluOpType
    Act = mybir.ActivationFunctionType

    nc = bacc.Bacc("TRN2", target_bir_lowering=False, debug=False)
    logits_d = nc.dram_tensor("logits", [NBLK, P, C * T], fp16, kind="ExternalInput")
    targets_d = nc.dram_tensor("targets", [NBLK, P, T], fp16, kind="ExternalInput")
    out_d = nc.dram_tensor("out", [2, MOUT, MOUT], fp32, kind="ExternalOutput")

    def body(tc, pools):
        lpool, tpool, epool, rpool, zpool, fpool, psump, fin, cpool = pools
        # iota-constant tile: ICB[m, c, g] = c for the bulk is_eq masks
        # (DVE memsets run during the initial DMA wait, so they are free;
        # shipping ICB via DMA instead was measured SLOWER - it delays
        # block 1's logits half on the gpsimd ring)
        icb = cpool.tile([P, NMM * MOUT], fp16)
        icb4 = icb[:].rearrange("p (m c g) -> p m c g", m=NMM, c=C)
        for c in range(1, C):
            nc.vector.memset(icb4[:, :, c, :], float(c))
        # two PSUM accumulators (even/odd blocks) so the first copy+DMA
        # overlaps the last block's compute
        accs = [psump.tile([MOUT, MOUT], fp32, name=f"acc{i}") for i in range(2)]
        outs = [fin.tile([MOUT, MOUT], fp32, name=f"outs{i}") for i in range(2)]
        for blk in range(NBLK):
            first, last = blk == 0, blk == NBLK - 1
            Lb = lpool.tile([P, C * T], fp16, tag="L")
            tt = tpool.tile([P, T], fp16, tag="t")
            ring_a = nc.sync if blk % 2 == 0 else nc.gpsimd
            ring_b = nc.gpsimd if blk % 2 == 0 else nc.sync
            la = logits_d.ap()[blk]
            # block 0: quarter DMAs/EXPs so the first exp starts sooner
            nq = 4 if first else 2
            QC = C // nq
            for q in range(nq):
                ring = ring_a if q % 2 == 0 else ring_b
                ring.dma_start(
                    Lb[:, q * QC * T : (q + 1) * QC * T],
                    la[:, q * QC * T : (q + 1) * QC * T],
                )
            ring_a.dma_start(tt[:], targets_d.ap()[blk])

            E = epool.tile([P, NMM * MOUT], fp16, tag="E")
            R = rpool.tile([P, NMM * MOUT], fp16, tag="R")
            zt = zpool.tile([P, 8 * T], fp16, tag="zt")
            Zf = fpool.tile([P, T], fp32, tag="Zf")
            Rf = fpool.tile([P, T], fp32, tag="Rf")
            rc = fpool.tile([P, T], fp16, tag="rc")
            E4 = E[:].rearrange("p (m c g) -> p m c g", m=NMM, c=C)
            R4 = R[:].rearrange("p (m c g) -> p m c g", m=NMM, c=C)
            Lg = Lb[:].rearrange("p (c m g) -> p c m g", c=C, g=G)
            tt4 = tt[:].rearrange("p (m o g) -> p m o g", o=1, g=G)
            rc3 = rc[:].rearrange("p (m g) -> p m g", g=G)
            rc4 = rc[:].rearrange("p (m o g) -> p m o g", o=1, g=G)
            z3 = zt[:].rearrange("p (s j) -> p s j", s=8)
            zg = zt[:].rearrange("p (s m g) -> p s m g", s=8, g=G)

            # e = exp(logits), one op per DMA granule
            for q in range(nq):
                nc.scalar.activation(
                    E4[:, :, q * QC : (q + 1) * QC, :].rearrange(
                        "p m c g -> p c m g"
                    ),
                    Lg[:, q * QC : (q + 1) * QC],
                    Act.Exp,
                )

            # Z tree fully on DVE (2x fp16 adds), fp32 tail for recip
            nc.vector.tensor_tensor(
                zg[:, 0:4],
                E4[:, :, 0:4, :].rearrange("p m s g -> p s m g"),
                E4[:, :, 4:8, :].rearrange("p m s g -> p s m g"),
                AL.add,
            )
            nc.vector.tensor_tensor(
                zg[:, 4:8],
                E4[:, :, 8:12, :].rearrange("p m s g -> p s m g"),
                E4[:, :, 12:16, :].rearrange("p m s g -> p s m g"),
                AL.add,
            )
            nc.vector.tensor_tensor(
                z3[:, 0:4, :], z3[:, 0:4, :], z3[:, 4:8, :], AL.add
            )
            nc.vector.tensor_tensor(
                z3[:, 0:2, :], z3[:, 0:2, :], z3[:, 2:4, :], AL.add
            )
            nc.vector.tensor_tensor(Zf[:], z3[:, 0, :], z3[:, 1, :], AL.add)
            nc.vector.reciprocal_approx_fast(Rf[:], Zf[:])
            nc.vector.tensor_copy(rc[:], Rf[:])

            # last block: split mask work + matmuls in m-halves so the
            # PE drain starts at the half mark
            acc = accs[blk % 2]
            mranges = [(0, NMM // 2), (NMM // 2, NMM)] if last else [(0, NMM)]
            for m0, m1 in mranges:
                ms = slice(m0, m1)
                # R slot 0 := plain r (probs_sum column)
                nc.vector.tensor_copy(R4[:, ms, 0, :], rc3[:, ms])
                # bulk masks: one 2x is_eq vs the iota tile, one 2x
                # broadcast multiply folds r in (both HW-verified 2x)
                tgt = R4[:, ms, 1:C, :]
                tin, _ = bass.broadcast_tensor_aps(tt4[:, ms], tgt)
                rin, _ = bass.broadcast_tensor_aps(rc4[:, ms], tgt)
                nc.vector.tensor_tensor(tgt, tin, icb4[:, ms, 1:C, :], AL.is_equal)
                nc.vector.tensor_tensor(tgt, tgt, rin, AL.mult)
                for m in range(m0, m1):
                    nc.tensor.matmul(
                        acc[:],
                        E[:, m * MOUT : (m + 1) * MOUT],
                        R[:, m * MOUT : (m + 1) * MOUT],
                        start=(blk < 2 and m == 0),
                        stop=(blk >= 2 and m == NMM - 1),
                    )
            if blk >= 2:
                # accumulator for this parity is complete: copy + ship
                nc.vector.tensor_copy(outs[blk % 2][:], acc[:])
                nc.sync.dma_start(out_d.ap()[blk % 2], outs[blk % 2][:])

    with tile.TileContext(nc) as tc:
        with (
            tc.tile_pool(name="lpool", bufs=2) as lpool,
            tc.tile_pool(name="tpool", bufs=2) as tpool,
            tc.tile_pool(name="epool", bufs=3) as epool,
            tc.tile_pool(name="rpool", bufs=3) as rpool,
            tc.tile_pool(name="zpool", bufs=2) as zpool,
            tc.tile_pool(name="fpool", bufs=2) as fpool,
            tc.tile_pool(name="psum", bufs=1, space="PSUM") as psump,
            tc.tile_pool(name="fin", bufs=1) as fin,
            tc.tile_pool(name="cpool", bufs=1) as cpool,
        ):
            body(tc, (lpool, tpool, epool, rpool, zpool, fpool, psump, fin, cpool))
    nc.compile()
    return nc


_NC_CACHE = {}


def _get_nc():
    if "nc" not in _NC_CACHE:
        _NC_CACHE["nc"] = build()
    return _NC_CACHE["nc"]


def stats_from_out(out_mat):
    """out[c1*8+g, c2*8+g] summed over g -> one 16x16 stats matrix."""
    M = out_mat.astype(np.float64).reshape(C, G, C, G)
    return np.einsum("agbg->ab", M)


def loss_from_stats(S_per_b):
    """S_per_b: (B, 16, 16) combined stats -> scalar loss.

    R slot 0 held plain r, slots 1..15 held (t==c)*r, so:
      probs_sum[c] = S[c, 0], counts[c] = sum_c1 S[c1, c] (c >= 1),
      inter[c] = S[c, c] (c >= 1). Class 0 is dice-excluded.
    """
    idx = np.arange(C)
    inter = S_per_b[:, idx, idx]          # (B, C); [*, 0] is garbage
    probs_sum = S_per_b[:, :, 0]          # (B, C)  sum_n e_c * r
    counts = S_per_b.sum(axis=1)          # (B, C); [*, 0] is garbage
    dice = (2.0 * inter + SMOOTH) / (probs_sum + counts + SMOOTH)
    mask = np.ones(C)
    mask[IGNORE_INDEX] = 0.0
    mean_dice = (dice * mask[None, :]).sum() / (B * (C - 1))
    return np.float32(1.0 - mean_dice)


def shard_inputs(logits, targets):
    """Core i gets batch i//4, d-slab i%4.

    Device layout (voxel n = p*(NBLK*T) + blk*T + j):
      logits  [NBLK, P, C, T] fp32
      targets [NBLK, P, T]    fp16
    """
    in_maps = []
    for i in range(NCORES):
        b, q = divmod(i, 4)
        lg = logits[b, :, q * DSH : (q + 1) * DSH].reshape(C, P, NBLK, T)
        lg = np.ascontiguousarray(lg.transpose(2, 1, 0, 3), dtype=np.float16)
        tg = targets[b, q * DSH : (q + 1) * DSH].reshape(P, NBLK, T)
        tg = np.ascontiguousarray(tg.transpose(1, 0, 2)).astype(np.float16)
        in_maps.append({"logits": lg.reshape(NBLK, P, C * T), "targets": tg})
    return in_maps


def kernel(logits, targets):
    logits = np.asarray(logits)
    targets = np.asarray(targets)
    nc = _get_nc()
    in_maps = shard_inputs(logits, targets)
    res = run_bass_kernel_spmd(nc, in_maps, list(range(NCORES))).results
    S = np.zeros((B, C, C), np.float64)
    for i in range(NCORES):
        om = res[i]["out"]
        S[i // 4] += stats_from_out(om[0]) + stats_from_out(om[1])
    return loss_from_stats(S)


# revision 31
# speedup vs baseline: 1.1091x; 1.0293x over previous
"""Memory-efficient Dice loss on 8 Trainium2 NeuronCores.

Full inputs:
  logits  (2, 16, 64, 128, 128) fp32
  targets (2, 64, 128, 128) int64  (values 0..15)
Output: scalar fp32 loss = 1 - mean_{b, c != 0} dice[b, c].

Sharding: 8 cores over (B=2) x (D quartered into 4 slabs of 16).
Each core reduces its shard to one 128x128 stats matrix; host combines
the tiny per-core stats and applies the dice formula.

Per-core math (voxels n, classes c), fp16 on-chip:
  e[n,c] = exp(logit[n,c]); Z[n] = sum_c e; r[n] = 1/Z
  R slot c>=1: (t==c) * r;  R slot 0: plain r
  PSUM-accumulated fp16 matmuls: S[c1,c2] = sum_n e[n,c1] * R[n,c2]
    diag(S)[c>=1]        = intersection
    S[:, 0]              = sum_n e_c1 * r = probs_sum  (exact column)
    sum_c1 S[c1, c>=1]   = sum_n (Z*r) * mask_c = counts  (Z*r == 1)
  Class 0 is dice-excluded (IGNORE_INDEX), so its mask is never needed.

Layout: "blocked chunk-major". E/R tiles hold element (chunk m, slot c,
lane g) at m*128 + c*8 + g: each of the 64 matmuls per block reads a
CONTIGUOUS 128-column slice (walrus requires 1-free-dim matmul
operands) and every elementwise op sees packed 8-lane fp16 runs.

Engine facts (HW-measured): DVE TT adds run 2x (0.56ns/elem); DVE
scalar_tensor_tensor gets NO fast mode (~600ns per 512-elem op, fp32
in1 free); GpSimd TT ops starve DVE when run concurrently (4-7x DVE
slowdown), so the whole per-block chain stays on DVE and GpSimd only
issues DMAs on its ring. PE overlaps LDWEIGHTS with MATMUL (~107ns per
128-col fp16 matmul). ACT exp runs 0.87ns/elem with 8-lane writes.

DMA: host pre-permutes logits to [nblk][p][c][j] fp16 (the kernel is
fp16 internally anyway, so the cast costs no accuracy headroom: final
rel err stays ~1e-4, tolerance is 2e-2) and targets to
[nblk][p][j] fp16; each block is two contiguous class-half dma_starts
split across the sync/gpsimd rings (block 0: quarters, to cut the
pipeline head). The last block's mask ops + matmuls are split in
m-halves so the PE drain starts at the half mark.
"""

import numpy as np

import concourse.bass as bass
import concourse.mybir as mybir
import concourse.tile as tile
from concourse import bacc
from concourse.bass_utils import run_bass_kernel_spmd

B, C, D, H, W = 2, 16, 64, 128, 128
P = 128            # SBUF partitions
NCORES = 8
DSH = D // 4       # d-planes per core
N = DSH * H * W    # voxels per core = 262144
G = 8              # packed chunk lanes per matmul
MOUT = C * G       # 128

NBLK = 4
T = N // (P * NBLK)         # voxel columns per block = 512
NMM = T // G                # matmuls per block = 64

SMOOTH = 1.0
IGNORE_INDEX = 0


def build():
    """Build the SPMD single-core Bass program."""
    fp32 = mybir.dt.float32
    fp16 = mybir.dt.float16
    AL = mybir.AluOpType
    Act = mybir.ActivationFunctionType

    nc = bacc.Bacc("TRN2", target_bir_lowering=False, debug=False)
    logits_d = nc.dram_tensor("logits", [NBLK, P, C * T], fp16, kind="ExternalInput")
    targets_d = nc.dram_tensor("targets", [NBLK, P, T], fp16, kind="ExternalInput")
    out_d = nc.dram_tensor("out", [2, MOUT, MOUT], fp32, kind="ExternalOutput")

    def body(tc, pools):
        lpool, tpool, epool, rpool, zpool, fpool, psump, fin, cpool = pools
        # iota-constant tile: ICB[m, c, g] = c for the bulk is_eq masks
        # (DVE memsets run during the initial DMA wait, so they are free;
        # shipping ICB via DMA instead was measured SLOWER - it delays
        # block 1's logits half on the gpsimd ring)
        icb = cpool.tile([P, NMM * MOUT], fp16)
        icb4 = icb[:].rearrange("p (m c g) -> p m c g", m=NMM, c=C)
        for c in range(1, C):
            nc.vector.memset(icb4[:, :, c, :], float(c))
        # two PSUM accumulators (even/odd blocks) so the first copy+DMA
        # overlaps the last block's compute
        accs = [psump.tile([MOUT, MOUT], fp32, name=f"acc{i}") for i in range(2)]
        outs = [fin.tile([MOUT, MOUT], fp32, name=f"outs{i}") for i in range(2)]
        for blk in range(NBLK):
            first, last = blk == 0, blk == NBLK - 1
            Lb = lpool.tile([P, C * T], fp16, tag="L")
            tt = tpool.tile([P, T], fp16, tag="t")
            ring_a = nc.sync if blk % 2 == 0 else nc.gpsimd
            ring_b = nc.gpsimd if blk % 2 == 0 else nc.sync
            la = logits_d.ap()[blk]
            # block 0: quarter DMAs/EXPs so the first exp starts sooner
            nq = 4 if first else 2
            QC = C // nq
            for q in range(nq):
                ring = ring_a if q % 2 == 0 else ring_b
                ring.dma_start(
                    Lb[:, q * QC * T : (q + 1) * QC * T],
                    la[:, q * QC * T : (q + 1) * QC * T],
                )
            ring_a.dma_start(tt[:], targets_d.ap()[blk])

            E = epool.tile([P, NMM * MOUT], fp16, tag="E")
            R = rpool.tile([P, NMM * MOUT], fp16, tag="R")
            zt = zpool.tile([P, 8 * T], fp16, tag="zt")
            Zf = fpool.tile([P, T], fp32, tag="Zf")
            Rf = fpool.tile([P, T], fp32, tag="Rf")
            E4 = E[:].rearrange("p (m c g) -> p m c g", m=NMM, c=C)
            R4 = R[:].rearrange("p (m c g) -> p m c g", m=NMM, c=C)
            Lg = Lb[:].rearrange("p (c m g) -> p c m g", c=C, g=G)
            tt4 = tt[:].rearrange("p (m o g) -> p m o g", o=1, g=G)
            Rf3 = Rf[:].rearrange("p (m g) -> p m g", g=G)
            z3 = zt[:].rearrange("p (s j) -> p s j", s=8)
            zg = zt[:].rearrange("p (s m g) -> p s m g", s=8, g=G)

            # bulk one-hot masks depend only on targets + icb, so they
            # are issued BEFORE the tree: block 0's masks fill the boot
            # ramp while DVE would otherwise idle waiting for exp
            tgt_all = R4[:, :, 1:C, :]
            tin_all, _ = bass.broadcast_tensor_aps(tt4[:], tgt_all)
            nc.vector.tensor_tensor(
                tgt_all, tin_all, icb4[:, :, 1:C, :], AL.is_equal
            )

            # e = exp(logits), one op per DMA granule
            for q in range(nq):
                nc.scalar.activation(
                    E4[:, :, q * QC : (q + 1) * QC, :].rearrange(
                        "p m c g -> p c m g"
                    ),
                    Lg[:, q * QC : (q + 1) * QC],
                    Act.Exp,
                )

            # Z tree fully on DVE (2x fp16 adds), fp32 tail for recip
            nc.vector.tensor_tensor(
                zg[:, 0:4],
                E4[:, :, 0:4, :].rearrange("p m s g -> p s m g"),
                E4[:, :, 4:8, :].rearrange("p m s g -> p s m g"),
                AL.add,
            )
            nc.vector.tensor_tensor(
                zg[:, 4:8],
                E4[:, :, 8:12, :].rearrange("p m s g -> p s m g"),
                E4[:, :, 12:16, :].rearrange("p m s g -> p s m g"),
                AL.add,
            )
            nc.vector.tensor_tensor(
                z3[:, 0:4, :], z3[:, 0:4, :], z3[:, 4:8, :], AL.add
            )
            nc.vector.tensor_tensor(
                z3[:, 0:2, :], z3[:, 0:2, :], z3[:, 2:4, :], AL.add
            )
            nc.vector.tensor_tensor(Zf[:], z3[:, 0, :], z3[:, 1, :], AL.add)
            nc.vector.reciprocal_approx_fast(Rf[:], Zf[:])
            # cast r straight into R slot 0 (probs_sum column); the fold
            # broadcasts from there, so no separate rc tile is needed
            nc.vector.tensor_copy(R4[:, :, 0, :], Rf3[:])

            # last block: split mask work + matmuls in m-halves so the
            # PE drain starts at the half mark
            acc = accs[blk % 2]
            mranges = [(0, NMM // 2), (NMM // 2, NMM)] if last else [(0, NMM)]
            for m0, m1 in mranges:
                ms = slice(m0, m1)
                # fold r into the pre-built masks (2x broadcast multiply
                # from R slot 0)
                tgt = R4[:, ms, 1:C, :]
                rin, _ = bass.broadcast_tensor_aps(R4[:, ms, 0:1, :], tgt)
                nc.vector.tensor_tensor(tgt, tgt, rin, AL.mult)
                for m in range(m0, m1):
                    nc.tensor.matmul(
                        acc[:],
                        E[:, m * MOUT : (m + 1) * MOUT],
                        R[:, m * MOUT : (m + 1) * MOUT],
                        start=(blk < 2 and m == 0),
                        stop=(blk >= 2 and m == NMM - 1),
                    )
            if blk >= 2:
                # accumulator for this parity is complete: copy + ship
                nc.vector.tensor_copy(outs[blk % 2][:], acc[:])
                nc.sync.dma_start(out_d.ap()[blk % 2], outs[blk % 2][:])

    with tile.TileContext(nc) as tc:
        with (
            tc.tile_pool(name="lpool", bufs=2) as lpool,
            tc.tile_pool(name="tpool", bufs=2) as tpool,
            tc.tile_pool(name="epool", bufs=3) as epool,
            tc.tile_pool(name="rpool", bufs=3) as rpool,
            tc.tile_pool(name="zpool", bufs=2) as zpool,
            tc.tile_pool(name="fpool", bufs=2) as fpool,
            tc.tile_pool(name="psum", bufs=1, space="PSUM") as psump,
            tc.tile_pool(name="fin", bufs=1) as fin,
            tc.tile_pool(name="cpool", bufs=1) as cpool,
        ):
            body(tc, (lpool, tpool, epool, rpool, zpool, fpool, psump, fin, cpool))
    nc.compile()
    return nc


_NC_CACHE = {}


def _get_nc():
    if "nc" not in _NC_CACHE:
        _NC_CACHE["nc"] = build()
    return _NC_CACHE["nc"]


def stats_from_out(out_mat):
    """out[c1*8+g, c2*8+g] summed over g -> one 16x16 stats matrix."""
    M = out_mat.astype(np.float64).reshape(C, G, C, G)
    return np.einsum("agbg->ab", M)


def loss_from_stats(S_per_b):
    """S_per_b: (B, 16, 16) combined stats -> scalar loss.

    R slot 0 held plain r, slots 1..15 held (t==c)*r, so:
      probs_sum[c] = S[c, 0], counts[c] = sum_c1 S[c1, c] (c >= 1),
      inter[c] = S[c, c] (c >= 1). Class 0 is dice-excluded.
    """
    idx = np.arange(C)
    inter = S_per_b[:, idx, idx]          # (B, C); [*, 0] is garbage
    probs_sum = S_per_b[:, :, 0]          # (B, C)  sum_n e_c * r
    counts = S_per_b.sum(axis=1)          # (B, C); [*, 0] is garbage
    dice = (2.0 * inter + SMOOTH) / (probs_sum + counts + SMOOTH)
    mask = np.ones(C)
    mask[IGNORE_INDEX] = 0.0
    mean_dice = (dice * mask[None, :]).sum() / (B * (C - 1))
    return np.float32(1.0 - mean_dice)


def shard_inputs(logits, targets):
    """Core i gets batch i//4, d-slab i%4.

    Device layout (voxel n = p*(NBLK*T) + blk*T + j):
      logits  [NBLK, P, C, T] fp32
      targets [NBLK, P, T]    fp16
    """
    in_maps = []
    for i in range(NCORES):
        b, q = divmod(i, 4)
        lg = logits[b, :, q * DSH : (q + 1) * DSH].reshape(C, P, NBLK, T)
        lg = np.ascontiguousarray(lg.transpose(2, 1, 0, 3), dtype=np.float16)
        tg = targets[b, q * DSH : (q + 1) * DSH].reshape(P, NBLK, T)
        tg = np.ascontiguousarray(tg.transpose(1, 0, 2)).astype(np.float16)
        in_maps.append({"logits": lg.reshape(NBLK, P, C * T), "targets": tg})
    return in_maps


def kernel(logits, targets):
    logits = np.asarray(logits)
    targets = np.asarray(targets)
    nc = _get_nc()
    in_maps = shard_inputs(logits, targets)
    res = run_bass_kernel_spmd(nc, in_maps, list(range(NCORES))).results
    S = np.zeros((B, C, C), np.float64)
    for i in range(NCORES):
        om = res[i]["out"]
        S[i // 4] += stats_from_out(om[0]) + stats_from_out(om[1])
    return loss_from_stats(S)
